# revision 7
# baseline (speedup 1.0000x reference)
"""Dot-product attention (no softmax) on 8 TRN2 NeuronCores.

out[b,h] = (q[b,h] @ k[b,h].T) @ v[b,h]  for q,k,v [B,H,L,D] = [2,16,2048,64] f32.

Strategy: matmul associativity -> out = q @ (k.T @ v). KV = k.T@v is [64,64]
per head, so the problem collapses from O(L^2 D) to O(L D^2) flops and becomes
purely memory bound (48 MiB in / 16 MiB out).

Sharding: the 32 (b,h) attention instances are independent; each of the 8
cores handles 4 consecutive heads of the flattened (b*h) axis. No collectives.

Per-core layout trick: a head's [2048, 64] tensor is viewed as [128, 16, 64]
(partition p holds rows 16p..16p+15, 4 KiB contiguous DRAM per partition, so
every DMA is fully coalesced). The KV reduction over L is order-independent,
and the same interleaved row mapping flows through transpose -> matmul ->
store unchanged.

Perf structure (v2): the kernel is paced by HBM DMA (~420 GB/s/core, 8.4 MiB
total). Everything else is hidden under the load stream:
- transposed q (qt) and KV are cast to bf16 inside the mandatory PSUM->SBUF
  copies, so the 32 out-matmuls run single-pass bf16 with fast weight load
  (fp32 matmuls double-pump: 2x LDWEIGHTS + 2x MATMUL each).
- per-head software pipeline T(h)/KV(h)/out(h-1) keeps the PE busy (HAM
  un-throttled at 2.4 GHz) and lets output stores stream during input loads
  instead of piling into a PE-bound tail.
- q of the last head is loaded last, in 4 chunks, with transpose->matmul->
  store per chunk, so the final stores trail the final load by <1us.
- all DMA on the sync HW queue (FIFO): loads first, stores drain behind them.
- PSUM->SBUF copies alternate scalar/vector engines (GpSimd and DMA have no
  PSUM port); stores of finished halves are just queued on sync.
"""

import sys

if "/opt/trn_rl_repo" not in sys.path:
    sys.path.insert(0, "/opt/trn_rl_repo")

from contextlib import ExitStack

import numpy as np

import concourse.bass as bass
import concourse.tile as tile
from concourse import bacc, mybir
from concourse.bass_utils import run_bass_kernel_spmd
from concourse.masks import make_identity

B, H, L, D = 2, 16, 2048, 64
N_CORES = 8
HPC = (B * H) // N_CORES  # heads per core = 4
P = 128
J = L // P  # 16 row-slots per partition
F32 = mybir.dt.float32
BF16 = mybir.dt.bfloat16


def _body(ctx: ExitStack, tc: tile.TileContext, o_d, q_d, k_d, v_d):
    nc = tc.nc

    const_pool = ctx.enter_context(tc.tile_pool(name="const", bufs=1))
    in_pool = ctx.enter_context(tc.tile_pool(name="in", bufs=4))
    qt_pool = ctx.enter_context(tc.tile_pool(name="qt", bufs=16))
    kv_pool = ctx.enter_context(tc.tile_pool(name="kv", bufs=4))
    out_pool = ctx.enter_context(tc.tile_pool(name="out", bufs=4))
    psum_o = ctx.enter_context(tc.tile_pool(name="psum_o", bufs=2, space="PSUM"))
    psum_t = ctx.enter_context(tc.tile_pool(name="psum_t", bufs=2, space="PSUM"))
    psum_kv = ctx.enter_context(tc.tile_pool(name="psum_kv", bufs=2, space="PSUM"))
    psum_s = ctx.enter_context(tc.tile_pool(name="psum_s", bufs=1, space="PSUM"))

    # GpSimd constants (no PSUM port on GpSimd, so it only ever touches SBUF).
    warm_in = const_pool.tile([P, 4 * P], BF16)
    nc.gpsimd.memset(warm_in[:], 0.0)

    ident = const_pool.tile([P, P], F32)
    make_identity(nc, ident[:])

    # ones_dbl[p, m] = 1 iff p == m (mod 64): one matmul against it both sums
    # the two column-tiled KV halves and replicates the result to partitions
    # 64..127 (the block-diag KV2 operand needs KV on both partition halves).
    ones_dbl = const_pool.tile([P, P], BF16)
    nc.gpsimd.memset(ones_dbl[:], 0.0)
    for off in (-64, 0, 64):
        nc.gpsimd.affine_select(
            out=ones_dbl[:],
            in_=ones_dbl[:],
            compare_op=mybir.AluOpType.not_equal,
            fill=1.0,
            base=-off,
            pattern=[[-1, P]],
            channel_multiplier=1,
        )

    # ---- input loads: all on the sync HW queue, issued back-to-back.
    # q3 is loaded LAST in 4 chunks so the tail (T3 -> out3 -> store3) is
    # paced by its own loads and the final store trails the final load.
    q_sbs, k_sbs, v_sbs = [], [], []
    for h in range(HPC):
        q_sbs.append(in_pool.tile([P, J, D], F32, tag="q", name=f"q{h}"))
        k_sbs.append(in_pool.tile([P, J, D], F32, tag="k", name=f"k{h}"))
        v_sbs.append(in_pool.tile([P, J, D], F32, tag="v", name=f"v{h}"))

    def load_q(h, lo, hi):
        qv = q_d[h].rearrange("(p j) d -> p j d", p=P)
        nc.sync.dma_start(q_sbs[h][:, lo:hi], qv[:, lo:hi])

    def load_k(h):
        nc.sync.dma_start(k_sbs[h][:], k_d[h].rearrange("(p j) d -> p j d", p=P))

    def load_v(h):
        nc.sync.dma_start(v_sbs[h][:], v_d[h].rearrange("(p j) d -> p j d", p=P))

    load_q(0, 0, J)
    load_k(0)
    load_v(0)
    for h in (1, 2):
        load_q(h, 0, J)
        load_k(h)
        load_v(h)
    load_k(3)
    load_v(3)
    for c in range(4):
        load_q(3, 4 * c, 4 * c + 4)

    # HAM warm-up: dense bf16 matmuls while the first DMAs are in flight, so
    # the PE clock un-throttles (4/8 -> 8/8) before real work starts. 12 MMs
    # (~5us at the cold clock) bridge the gap until k0/v0 land, so the
    # activity monitor never sees an idle window mid-kernel.
    warm_ps = psum_o.tile([P, 8, D], F32, tag="o_ps", name="warm_ps")
    for _ in range(12):
        nc.tensor.matmul(
            warm_ps[:], warm_in[:, 0:P], warm_in[:], start=True, stop=True
        )

    # PSUM->SBUF copies alternate scalar/vector (they can run in parallel on
    # different PSUM banks).
    cp_state = [0]

    def copy(dst, src, eng=None):
        if eng is None:
            cp_state[0] ^= 1
            eng = "s" if cp_state[0] else "v"
        if eng == "s":
            nc.scalar.activation(dst, src, mybir.ActivationFunctionType.Copy)
        else:
            nc.vector.tensor_copy(dst, src)

    qts = [[None] * (J // 4) for _ in range(HPC)]
    kv2s = [None] * HPC

    def emit_T(h, a, eng=None):
        # transpose q pairs 2a, 2a+1 (slots 4a..4a+3) -> one bf16 qt tile
        q_sb = q_sbs[h]
        tps = psum_t.tile([P, 2, P], F32, tag="qt_ps")
        for i, p in enumerate((2 * a, 2 * a + 1)):
            nc.tensor.transpose(tps[:, i], q_sb[:, 2 * p : 2 * p + 2], ident[:])
        qt = qt_pool.tile([P, 2, P], BF16, tag="qt", name=f"qt{h}_{a}")
        copy(qt[:], tps[:], eng)
        qts[h][a] = qt

    def emit_KV(h):
        # KV = k.T @ v, column-tiled: even j-slots accumulate into PE columns
        # 0..63 (psum partitions 0..63), odd slots into columns 64..127.
        k_sb, v_sb = k_sbs[h], v_sbs[h]
        kv_ps = psum_kv.tile([P, D], F32, tag="kv_ps")
        for jp in range(J // 2):
            nc.tensor.matmul(
                kv_ps[0:D],
                k_sb[:, 2 * jp],
                v_sb[:, 2 * jp],
                start=(jp == 0),
                stop=(jp == J // 2 - 1),
                tile_position=(0, 0),
                skip_group_check=True,
            )
            nc.tensor.matmul(
                kv_ps[D : 2 * D],
                k_sb[:, 2 * jp + 1],
                v_sb[:, 2 * jp + 1],
                start=(jp == 0),
                stop=(jp == J // 2 - 1),
                tile_position=(0, D),
                skip_group_check=True,
            )
        return kv_ps

    def emit_kvfix(h, kv_ps):
        # sum the two column halves + replicate to partitions 64..127, then
        # lay out as block-diag KV2 in bf16.
        kv_raw = kv_pool.tile([P, D], BF16, tag="kv_raw", name=f"kvr{h}")
        copy(kv_raw[:], kv_ps[:])
        kv_st = psum_s.tile([P, D], F32, tag="kv_st", name=f"kvs{h}")
        nc.tensor.matmul(kv_st[:], ones_dbl[:], kv_raw[:], start=True, stop=True)
        kv2 = kv_pool.tile([P, 2, D], BF16, tag="kv2", name=f"kv2_{h}")
        nc.gpsimd.memset(kv2[:], 0.0)
        copy(kv2[0:D, 0], kv_st[0:D])
        copy(kv2[D : 2 * D, 1], kv_st[D : 2 * D])
        kv2s[h] = kv2

    out_sbs = [
        out_pool.tile([P, J, D], F32, tag="o", name=f"o{h}") for h in range(HPC)
    ]

    def emit_out_half(h, half):
        # 4 pair-matmuls (bf16, N=128) -> one [128, 8, 64] copy -> 256 KiB
        # store. Stores go on GpSimd's (software) DMA queue: its ~2us
        # issue->transfer latency is hidden mid-stream, and it keeps the sync
        # HW queue free of store issues.
        ops = psum_o.tile([P, 8, D], F32, tag="o_ps")
        for i in range(4):
            p = 4 * half + i
            nc.tensor.matmul(
                ops[:, 2 * i : 2 * i + 2],
                qts[h][p // 2][:, p % 2],
                kv2s[h][:],
                start=True,
                stop=True,
            )
        sl = slice(8 * half, 8 * half + 8)
        copy(out_sbs[h][:, sl], ops[:])
        ov = o_d[h].rearrange("(p j) d -> p j d", p=P)
        nc.gpsimd.dma_start(ov[:, sl], out_sbs[h][:, sl])

    def emit_out_quarter(h, c, cp_eng, st_eng):
        # last head: 2 pair-matmuls -> one [128, 4, 64] copy -> 128 KiB store.
        # Copies alternate vector/scalar explicitly so the last chunk's copy
        # never queues behind the previous chunks'; the last stores issue on
        # the sync HW queue (low latency) in parallel with gpsimd's.
        ops = psum_o.tile([P, 8, D], F32, tag="o_ps")
        for i, p in enumerate((2 * c, 2 * c + 1)):
            nc.tensor.matmul(
                ops[:, 2 * i : 2 * i + 2],
                qts[h][p // 2][:, p % 2],
                kv2s[h][:],
                start=True,
                stop=True,
            )
        sl = slice(4 * c, 4 * c + 4)
        copy(out_sbs[h][:, sl], ops[:, 0:4], cp_eng)
        ov = o_d[h].rearrange("(p j) d -> p j d", p=P)
        if st_eng == "sync":
            nc.sync.dma_start(ov[:, sl], out_sbs[h][:, sl])
        else:
            nc.gpsimd.dma_start(ov[:, sl], out_sbs[h][:, sl])

    # ---- software pipeline: PE program order matches data arrival order so
    # the PE never head-of-line blocks: T(h) fills the gap while the kvfix
    # chain of head h-1 completes on scalar/vector, then out(h-1) runs.
    for a in range(4):
        emit_T(0, a)
    emit_kvfix(0, emit_KV(0))

    for h in (1, 2):
        for a in range(4):
            emit_T(h, a)
        emit_out_half(h - 1, 0)
        emit_out_half(h - 1, 1)
        emit_kvfix(h, emit_KV(h))

    emit_out_half(2, 0)
    emit_out_half(2, 1)
    emit_kvfix(3, emit_KV(3))

    # tail: per q3 chunk, transpose -> out -> store; interleaved so each
    # chunk's qt copy hides under the next chunk's transposes. Copy engines
    # pinned so consecutive chunks never share an engine back-to-back.
    emit_T(3, 0, "s")
    emit_T(3, 1, "v")
    emit_out_quarter(3, 0, "v", "gpsimd")
    emit_T(3, 2, "s")
    emit_out_quarter(3, 1, "s", "sync")
    emit_out_quarter(3, 2, "v", "gpsimd")
    emit_T(3, 3, "v")
    emit_out_quarter(3, 3, "s", "sync")


def build():
    nc = bacc.Bacc("TRN2", target_bir_lowering=False, debug=False)
    q_d = nc.dram_tensor("q", [HPC, L, D], F32, kind="ExternalInput").ap()
    k_d = nc.dram_tensor("k", [HPC, L, D], F32, kind="ExternalInput").ap()
    v_d = nc.dram_tensor("v", [HPC, L, D], F32, kind="ExternalInput").ap()
    o_d = nc.dram_tensor("out", [HPC, L, D], F32, kind="ExternalOutput").ap()
    with tile.TileContext(nc) as tc, ExitStack() as ctx:
        _body(ctx, tc, o_d, q_d, k_d, v_d)
    nc.compile()
    return nc


_NC = None


def _get_nc():
    global _NC
    if _NC is None:
        _NC = build()
    return _NC


def make_in_maps(q, k, v):
    qf = np.ascontiguousarray(np.asarray(q, dtype=np.float32).reshape(B * H, L, D))
    kf = np.ascontiguousarray(np.asarray(k, dtype=np.float32).reshape(B * H, L, D))
    vf = np.ascontiguousarray(np.asarray(v, dtype=np.float32).reshape(B * H, L, D))
    return [
        {
            "q": np.ascontiguousarray(qf[c * HPC : (c + 1) * HPC]),
            "k": np.ascontiguousarray(kf[c * HPC : (c + 1) * HPC]),
            "v": np.ascontiguousarray(vf[c * HPC : (c + 1) * HPC]),
        }
        for c in range(N_CORES)
    ]


def run_sharded(q, k, v, **spmd_kwargs):
    """Run on all 8 cores; returns (full_output, BassKernelResults)."""
    nc = _get_nc()
    res = run_bass_kernel_spmd(
        nc, make_in_maps(q, k, v), core_ids=list(range(N_CORES)), **spmd_kwargs
    )
    shards = [np.asarray(res.results[c]["out"]) for c in range(N_CORES)]
    out = np.concatenate(shards, axis=0).reshape(B, H, L, D).astype(np.float32)
    return out, res


def kernel(q, k, v):
    out, _ = run_sharded(q, k, v)
    return out


# revision 11
# speedup vs baseline: 1.0881x; 1.0881x over previous
"""Dot-product attention (no softmax) on 8 TRN2 NeuronCores.

out[b,h] = (q[b,h] @ k[b,h].T) @ v[b,h]  for q,k,v [B,H,L,D] = [2,16,2048,64] f32.

Strategy: matmul associativity -> out = q @ (k.T @ v). KV = k.T@v is [64,64]
per head, so the problem collapses from O(L^2 D) to O(L D^2) flops and becomes
purely memory bound (48 MiB in / 16 MiB out).

Sharding: the 32 (b,h) attention instances are independent; each of the 8
cores handles 4 consecutive heads of the flattened (b*h) axis. No collectives.

Per-core layout trick: a head's [2048, 64] tensor is viewed as [128, 16, 64]
(partition p holds rows 16p..16p+15, 4 KiB contiguous DRAM per partition, so
every DMA is fully coalesced). The KV reduction over L is order-independent,
and the same interleaved row mapping flows through transpose -> matmul ->
store unchanged.

Perf structure (v2): the kernel is paced by HBM DMA (~420 GB/s/core, 8.4 MiB
total). Everything else is hidden under the load stream:
- transposed q (qt) and KV are cast to bf16 inside the mandatory PSUM->SBUF
  copies, so the 32 out-matmuls run single-pass bf16 with fast weight load
  (fp32 matmuls double-pump: 2x LDWEIGHTS + 2x MATMUL each).
- per-head software pipeline T(h)/KV(h)/out(h-1) keeps the PE busy (HAM
  un-throttled at 2.4 GHz) and lets output stores stream during input loads
  instead of piling into a PE-bound tail.
- q of the last head is loaded last, in 4 chunks, with transpose->matmul->
  store per chunk, so the final stores trail the final load by <1us.
- all DMA on the sync HW queue (FIFO): loads first, stores drain behind them.
- PSUM->SBUF copies alternate scalar/vector engines (GpSimd and DMA have no
  PSUM port); stores of finished halves are just queued on sync.
"""

import sys

if "/opt/trn_rl_repo" not in sys.path:
    sys.path.insert(0, "/opt/trn_rl_repo")

from contextlib import ExitStack

import numpy as np

import concourse.bass as bass
import concourse.tile as tile
from concourse import bacc, mybir
from concourse.bass_utils import run_bass_kernel_spmd
from concourse.masks import make_identity

B, H, L, D = 2, 16, 2048, 64
N_CORES = 8
HPC = (B * H) // N_CORES  # heads per core = 4
P = 128
J = L // P  # 16 row-slots per partition
F32 = mybir.dt.float32
BF16 = mybir.dt.bfloat16


def _body(ctx: ExitStack, tc: tile.TileContext, o_d, q_d, k_d, v_d):
    nc = tc.nc

    const_pool = ctx.enter_context(tc.tile_pool(name="const", bufs=1))
    in_pool = ctx.enter_context(tc.tile_pool(name="in", bufs=4))
    qt_pool = ctx.enter_context(tc.tile_pool(name="qt", bufs=16))
    kv_pool = ctx.enter_context(tc.tile_pool(name="kv", bufs=4))
    out_pool = ctx.enter_context(tc.tile_pool(name="out", bufs=4))
    psum_o = ctx.enter_context(tc.tile_pool(name="psum_o", bufs=2, space="PSUM"))
    psum_t = ctx.enter_context(tc.tile_pool(name="psum_t", bufs=2, space="PSUM"))
    psum_kv = ctx.enter_context(tc.tile_pool(name="psum_kv", bufs=2, space="PSUM"))
    psum_s = ctx.enter_context(tc.tile_pool(name="psum_s", bufs=1, space="PSUM"))

    # GpSimd constants (no PSUM port on GpSimd, so it only ever touches SBUF).
    warm_in = const_pool.tile([P, 4 * P], BF16)
    nc.gpsimd.memset(warm_in[:], 0.0)

    ident = const_pool.tile([P, P], F32)
    make_identity(nc, ident[:])

    # ones_dbl[p, m] = 1 iff p == m (mod 64): one matmul against it both sums
    # the two column-tiled KV halves and replicates the result to partitions
    # 64..127 (the block-diag KV2 operand needs KV on both partition halves).
    ones_dbl = const_pool.tile([P, P], BF16)
    nc.gpsimd.memset(ones_dbl[:], 0.0)
    for off in (-64, 0, 64):
        nc.gpsimd.affine_select(
            out=ones_dbl[:],
            in_=ones_dbl[:],
            compare_op=mybir.AluOpType.not_equal,
            fill=1.0,
            base=-off,
            pattern=[[-1, P]],
            channel_multiplier=1,
        )

    # ---- input loads: all on the sync HW queue, issued back-to-back.
    # q3 is loaded LAST in 4 chunks so the tail (T3 -> out3 -> store3) is
    # paced by its own loads and the final store trails the final load.
    q_sbs, k_sbs, v_sbs = [], [], []
    for h in range(HPC):
        q_sbs.append(in_pool.tile([P, J, D], F32, tag="q", name=f"q{h}"))
        k_sbs.append(in_pool.tile([P, J, D], F32, tag="k", name=f"k{h}"))
        v_sbs.append(in_pool.tile([P, J, D], F32, tag="v", name=f"v{h}"))

    def load_q(h, lo, hi):
        qv = q_d[h].rearrange("(p j) d -> p j d", p=P)
        nc.sync.dma_start(q_sbs[h][:, lo:hi], qv[:, lo:hi])

    def load_k(h):
        nc.sync.dma_start(k_sbs[h][:], k_d[h].rearrange("(p j) d -> p j d", p=P))

    def load_v(h):
        nc.sync.dma_start(v_sbs[h][:], v_d[h].rearrange("(p j) d -> p j d", p=P))

    load_q(0, 0, J)
    load_k(0)
    load_v(0)
    for h in (1, 2):
        load_q(h, 0, J)
        load_k(h)
        load_v(h)
    load_k(3)
    load_v(3)
    for c in range(4):
        load_q(3, 4 * c, 4 * c + 4)

    # HAM warm-up: dense bf16 matmuls while the first DMAs are in flight, so
    # the PE clock un-throttles (4/8 -> 8/8) before real work starts. 12 MMs
    # (~5us at the cold clock) bridge the gap until k0/v0 land, so the
    # activity monitor never sees an idle window mid-kernel.
    warm_ps = psum_o.tile([P, 8, D], F32, tag="o_ps", name="warm_ps")
    for _ in range(12):
        nc.tensor.matmul(
            warm_ps[:], warm_in[:, 0:P], warm_in[:], start=True, stop=True
        )

    # PSUM->SBUF copies alternate scalar/vector (they can run in parallel on
    # different PSUM banks).
    cp_state = [0]

    def copy(dst, src, eng=None):
        if eng is None:
            cp_state[0] ^= 1
            eng = "s" if cp_state[0] else "v"
        if eng == "s":
            nc.scalar.activation(dst, src, mybir.ActivationFunctionType.Copy)
        else:
            nc.vector.tensor_copy(dst, src)

    qts = [[None] * (J // 4) for _ in range(HPC)]
    # kv2 block-diag tiles: memset all four upfront on gpsimd (they depend on
    # nothing) so no head's KV fixup ever queues behind other gpsimd work.
    kv2s = []
    for h in range(HPC):
        kv2 = kv_pool.tile([P, 2, D], BF16, tag="kv2", name=f"kv2_{h}")
        nc.gpsimd.memset(kv2[:], 0.0)
        kv2s.append(kv2)

    def emit_T(h, a, eng=None):
        # transpose q pairs 2a, 2a+1 (slots 4a..4a+3) -> one bf16 qt tile
        q_sb = q_sbs[h]
        tps = psum_t.tile([P, 2, P], F32, tag="qt_ps")
        for i, p in enumerate((2 * a, 2 * a + 1)):
            nc.tensor.transpose(tps[:, i], q_sb[:, 2 * p : 2 * p + 2], ident[:])
        qt = qt_pool.tile([P, 2, P], BF16, tag="qt", name=f"qt{h}_{a}")
        copy(qt[:], tps[:], eng)
        qts[h][a] = qt

    def emit_KV(h):
        # KV = k.T @ v, column-tiled: even j-slots accumulate into PE columns
        # 0..63 (psum partitions 0..63), odd slots into columns 64..127.
        k_sb, v_sb = k_sbs[h], v_sbs[h]
        kv_ps = psum_kv.tile([P, D], F32, tag="kv_ps")
        for jp in range(J // 2):
            nc.tensor.matmul(
                kv_ps[0:D],
                k_sb[:, 2 * jp],
                v_sb[:, 2 * jp],
                start=(jp == 0),
                stop=(jp == J // 2 - 1),
                tile_position=(0, 0),
                skip_group_check=True,
            )
            nc.tensor.matmul(
                kv_ps[D : 2 * D],
                k_sb[:, 2 * jp + 1],
                v_sb[:, 2 * jp + 1],
                start=(jp == 0),
                stop=(jp == J // 2 - 1),
                tile_position=(0, D),
                skip_group_check=True,
            )
        return kv_ps

    def emit_kvfix(h, kv_ps):
        # sum the two column halves + replicate to partitions 64..127, then
        # lay out as block-diag KV2 in bf16.
        kv_raw = kv_pool.tile([P, D], BF16, tag="kv_raw", name=f"kvr{h}")
        copy(kv_raw[:], kv_ps[:])
        kv_st = psum_s.tile([P, D], F32, tag="kv_st", name=f"kvs{h}")
        nc.tensor.matmul(kv_st[:], ones_dbl[:], kv_raw[:], start=True, stop=True)
        kv2 = kv2s[h]
        copy(kv2[0:D, 0], kv_st[0:D])
        copy(kv2[D : 2 * D, 1], kv_st[D : 2 * D])

    out_sbs = [
        out_pool.tile([P, J, D], F32, tag="o", name=f"o{h}") for h in range(HPC)
    ]

    def emit_out_half(h, half):
        # 4 pair-matmuls (bf16, N=128) -> one [128, 8, 64] copy -> 256 KiB
        # store on the sync HW queue (FIFO behind the loads, which are all
        # issued by then; gpsimd's software queue is far too slow for this).
        ops = psum_o.tile([P, 8, D], F32, tag="o_ps")
        for i in range(4):
            p = 4 * half + i
            nc.tensor.matmul(
                ops[:, 2 * i : 2 * i + 2],
                qts[h][p // 2][:, p % 2],
                kv2s[h][:],
                start=True,
                stop=True,
            )
        sl = slice(8 * half, 8 * half + 8)
        copy(out_sbs[h][:, sl], ops[:])
        ov = o_d[h].rearrange("(p j) d -> p j d", p=P)
        nc.sync.dma_start(ov[:, sl], out_sbs[h][:, sl])

    def emit_out_quarter(h, c, cp_eng, st_eng):
        # last head: 2 pair-matmuls -> one [128, 4, 64] copy -> 128 KiB store.
        # Copies alternate vector/scalar explicitly so the last chunk's copy
        # never queues behind the previous chunks'; store issues alternate
        # between the two HW DGE queues (sync, scalar) so they overlap.
        ops = psum_o.tile([P, 8, D], F32, tag="o_ps")
        for i, p in enumerate((2 * c, 2 * c + 1)):
            nc.tensor.matmul(
                ops[:, 2 * i : 2 * i + 2],
                qts[h][p // 2][:, p % 2],
                kv2s[h][:],
                start=True,
                stop=True,
            )
        sl = slice(4 * c, 4 * c + 4)
        copy(out_sbs[h][:, sl], ops[:, 0:4], cp_eng)
        ov = o_d[h].rearrange("(p j) d -> p j d", p=P)
        if st_eng == "sync":
            nc.sync.dma_start(ov[:, sl], out_sbs[h][:, sl])
        else:
            nc.scalar.dma_start(ov[:, sl], out_sbs[h][:, sl])

    # ---- software pipeline: PE program order matches data arrival order so
    # the PE never head-of-line blocks: T(h) fills the gap while the kvfix
    # chain of head h-1 completes on scalar/vector, then out(h-1) runs.
    for a in range(4):
        emit_T(0, a)
    emit_kvfix(0, emit_KV(0))

    for h in (1, 2):
        for a in range(4):
            emit_T(h, a)
        emit_out_half(h - 1, 0)
        emit_out_half(h - 1, 1)
        emit_kvfix(h, emit_KV(h))

    emit_out_half(2, 0)
    emit_out_half(2, 1)
    emit_kvfix(3, emit_KV(3))

    # tail: per q3 chunk, transpose -> out -> store; interleaved so each
    # chunk's qt copy hides under the next chunk's transposes. Copy engines
    # pinned so consecutive chunks never share an engine back-to-back.
    emit_T(3, 0, "s")
    emit_T(3, 1, "v")
    emit_out_quarter(3, 0, "v", "scalar")
    emit_T(3, 2, "s")
    emit_out_quarter(3, 1, "s", "sync")
    emit_out_quarter(3, 2, "v", "scalar")
    emit_T(3, 3, "v")
    emit_out_quarter(3, 3, "s", "sync")


def build():
    nc = bacc.Bacc("TRN2", target_bir_lowering=False, debug=False)
    q_d = nc.dram_tensor("q", [HPC, L, D], F32, kind="ExternalInput").ap()
    k_d = nc.dram_tensor("k", [HPC, L, D], F32, kind="ExternalInput").ap()
    v_d = nc.dram_tensor("v", [HPC, L, D], F32, kind="ExternalInput").ap()
    o_d = nc.dram_tensor("out", [HPC, L, D], F32, kind="ExternalOutput").ap()
    with tile.TileContext(nc) as tc, ExitStack() as ctx:
        _body(ctx, tc, o_d, q_d, k_d, v_d)
    nc.compile()
    return nc


_NC = None


def _get_nc():
    global _NC
    if _NC is None:
        _NC = build()
    return _NC


def make_in_maps(q, k, v):
    qf = np.ascontiguousarray(np.asarray(q, dtype=np.float32).reshape(B * H, L, D))
    kf = np.ascontiguousarray(np.asarray(k, dtype=np.float32).reshape(B * H, L, D))
    vf = np.ascontiguousarray(np.asarray(v, dtype=np.float32).reshape(B * H, L, D))
    return [
        {
            "q": np.ascontiguousarray(qf[c * HPC : (c + 1) * HPC]),
            "k": np.ascontiguousarray(kf[c * HPC : (c + 1) * HPC]),
            "v": np.ascontiguousarray(vf[c * HPC : (c + 1) * HPC]),
        }
        for c in range(N_CORES)
    ]


def run_sharded(q, k, v, **spmd_kwargs):
    """Run on all 8 cores; returns (full_output, BassKernelResults)."""
    nc = _get_nc()
    res = run_bass_kernel_spmd(
        nc, make_in_maps(q, k, v), core_ids=list(range(N_CORES)), **spmd_kwargs
    )
    shards = [np.asarray(res.results[c]["out"]) for c in range(N_CORES)]
    out = np.concatenate(shards, axis=0).reshape(B, H, L, D).astype(np.float32)
    return out, res


def kernel(q, k, v):
    out, _ = run_sharded(q, k, v)
    return out
